# revision 2
# baseline (speedup 1.0000x reference)
"""LoRA Linear kernel for Trainium2, 8-core tensor-parallel.

out = x @ W^T + b + 2.0 * ((x @ lora_B^T) @ lora_A^T)

Sharding: W / lora_A / b row-sharded (out_features) across 8 cores;
x and lora_B replicated. Host concatenates per-core output shards.

Per-core compute (all fp32 data, matmuls in float32r):
  - W_shard^T  pre-transposed on PE once, SBUF-resident  [4096k, 512o]
  - x streamed per 128-token tile, PE-transposed to x^T tiles
  - main MM:  psum[t128, o512] += xT[k128,t128].T @ WT[k128,o512]  (32 k-blocks)
  - xr^T[16, 512t] = sum_k loraB^T[k,16].T @ xT[k, t512]  (per 4-t-tile group)
  - stage2 MM (K=17, bias folded via ones-row):
      psum += [xr^T; 1]^T @ [2*lora_A^T; b]
"""

import numpy as np

N_CORES = 8
B_DIM, S_DIM, D_IN, D_OUT = 4, 2048, 4096, 4096
T = B_DIM * S_DIM          # 8192 tokens
O_SHARD = D_OUT // N_CORES  # 512
R = 16
P = 128
KB = D_IN // P             # 32 k-blocks
TT = T // P                # 64 token tiles
GROUP = 4                  # t-tiles per xr group (N=512 for xr matmuls)
NG = TT // GROUP           # 16 groups
import os as _os
NG_OVERRIDE = int(_os.environ.get("KERNEL_NG", "0"))
if NG_OVERRIDE:
    NG = NG_OVERRIDE

_CACHE = {}


def _build_nc():
    import concourse.bacc as bacc
    import concourse.mybir as mybir
    import concourse.tile as tile
    from concourse.masks import make_identity

    F32 = mybir.dt.float32
    F32R = mybir.dt.float32r

    nc = bacc.Bacc(target_bir_lowering=False)
    x_d = nc.dram_tensor("x", [T, D_IN], F32, kind="ExternalInput")
    w_d = nc.dram_tensor("w", [O_SHARD, D_IN], F32, kind="ExternalInput")
    b_d = nc.dram_tensor("b", [1, O_SHARD], F32, kind="ExternalInput")
    la_d = nc.dram_tensor("la", [O_SHARD, R], F32, kind="ExternalInput")
    lb_d = nc.dram_tensor("lb", [R, D_IN], F32, kind="ExternalInput")
    out_d = nc.dram_tensor("out", [T, O_SHARD], F32, kind="ExternalOutput")

    x_t = x_d[:].rearrange("(tt p) k -> p tt k", p=P)      # [128, 64, 4096]
    out_t = out_d[:].rearrange("(tt p) o -> p tt o", p=P)  # [128, 64, 512]

    with tile.TileContext(nc) as tc:
        with (
            tc.tile_pool(name="const", bufs=1) as const,
            tc.tile_pool(name="xin", bufs=2) as xin,
            tc.tile_pool(name="xt", bufs=1) as xtp,
            tc.tile_pool(name="osb", bufs=3) as osb_pool,
            tc.tile_pool(name="xr", bufs=2) as xrp,
            tc.tile_pool(name="ps_t", bufs=2, space="PSUM") as ps_t,
            tc.tile_pool(name="ps_o", bufs=2, space="PSUM") as ps_o,
            tc.tile_pool(name="ps_r", bufs=2, space="PSUM") as ps_r,
        ):
            ident = const.tile([P, P], F32)
            make_identity(nc, ident)

            # ---- resident weights ----
            wt = const.tile([P, KB, O_SHARD], F32)     # W^T  [128k, kb, 512o]
            lbt = const.tile([P, KB, R], F32)          # loraB^T [128k, kb, 16r]
            lat = const.tile([R, O_SHARD], F32)        # 2*lora_A^T
            b_bcast = const.tile([P, O_SHARD], F32)    # bias broadcast to 128 rows

            # W^T setup: stream W shard in 4 o-strips of [128, 4096]
            for oi in range(4):
                ws = xin.tile([P, D_IN], F32, tag="xs")
                nc.sync.dma_start(
                    ws, w_d[:].rearrange("(oi p) k -> p oi k", p=P)[:, oi, :]
                )
                for j8 in range(KB // 4):
                    pst = ps_t.tile([P, 4, P], F32, tag="pst")
                    for u in range(4):
                        j = j8 * 4 + u
                        nc.tensor.transpose(
                            pst[:, u, :], ws[:, j * P:(j + 1) * P], ident
                        )
                    nc.any.tensor_copy(
                        out=wt[:, j8 * 4:(j8 + 1) * 4,
                               oi * P:(oi + 1) * P].bitcast(F32R),
                        in_=pst,
                    )

            # loraB^T: lb [16, 4096] -> [128k, kb, 16]
            lbs = xin.tile([P, D_IN], F32, tag="xs")
            nc.sync.dma_start(lbs[:R, :], lb_d[:])
            psb = ps_r.tile([P, KB * R], F32, tag="psb")
            for j in range(KB):
                nc.tensor.transpose(
                    psb[:, j * R:(j + 1) * R],
                    lbs[:R, j * P:(j + 1) * P],
                    ident[:R, :R],
                )
            nc.any.tensor_copy(
                out=lbt[:].bitcast(F32R),
                in_=psb.rearrange("p (j r) -> p j r", j=KB),
            )

            # lora_A^T * 2  plus bias row
            las = xin.tile([P, 4 * R], F32, tag="las")
            nc.sync.dma_start(
                las.rearrange("p (oi r) -> p oi r", oi=4),
                la_d[:].rearrange("(oi p) r -> p oi r", p=P),
            )
            psa = ps_r.tile([P, 4 * P], F32, tag="psb")
            for oi in range(4):
                nc.tensor.transpose(
                    psa[:R, oi * P:(oi + 1) * P],
                    las[:, oi * R:(oi + 1) * R],
                    ident,
                )
            nc.scalar.mul(lat[:].bitcast(F32R), psa[:R, :], 2.0)

            # bias broadcast: ones[128,1] x b[1,512] via K=1 matmul
            ones_col = const.tile([1, P], F32)
            nc.any.memset(ones_col[:, :], 1.0)
            b_sb = const.tile([1, O_SHARD], F32)
            nc.sync.dma_start(b_sb, b_d[:])
            psbb = ps_o.tile([P, O_SHARD], F32, tag="pso")
            nc.tensor.matmul(psbb, ones_col, b_sb, start=True, stop=True)
            nc.any.tensor_copy(out=b_bcast, in_=psbb)

            # ---- main loop ----
            for g in range(NG):
                xt = xtp.tile([P, KB, GROUP * P], F32, tag="xt")
                for ti in range(GROUP):
                    tt = g * GROUP + ti
                    xs = xin.tile([P, D_IN], F32, tag="xs")
                    nc.sync.dma_start(xs, x_t[:, tt, :])
                    for j8 in range(KB // 4):
                        pst = ps_t.tile([P, 4, P], F32, tag="pst")
                        for u in range(4):
                            j = j8 * 4 + u
                            nc.tensor.transpose(
                                pst[:, u, :], xs[:, j * P:(j + 1) * P], ident
                            )
                        nc.any.tensor_copy(
                            out=xt[:, j8 * 4:(j8 + 1) * 4,
                                   ti * P:(ti + 1) * P].bitcast(F32R),
                            in_=pst,
                        )

                # xr^T for the whole group: [16, 512]
                psr = ps_r.tile([R, GROUP * P], F32, tag="psr")
                for j in range(KB):
                    nc.tensor.matmul(
                        psr,
                        lbt[:, j, :].bitcast(F32R),
                        xt[:, j, :].bitcast(F32R),
                        start=(j == 0),
                        stop=(j == KB - 1),
                    )
                xr_sb = xrp.tile([R, GROUP * P], F32, tag="xra")
                nc.any.tensor_copy(out=xr_sb[:, :].bitcast(F32R), in_=psr)

                for ti in range(GROUP):
                    tt = g * GROUP + ti
                    pso = ps_o.tile([P, O_SHARD], F32, tag="pso")
                    for j in range(KB):
                        nc.tensor.matmul(
                            pso,
                            xt[:, j, ti * P:(ti + 1) * P].bitcast(F32R),
                            wt[:, j, :].bitcast(F32R),
                            start=(j == 0),
                            stop=False,
                        )
                    nc.tensor.matmul(
                        pso,
                        xr_sb[:, ti * P:(ti + 1) * P].bitcast(F32R),
                        lat[:].bitcast(F32R),
                        start=False,
                        stop=True,
                    )
                    osb = osb_pool.tile([P, O_SHARD], F32, tag="osb")
                    nc.vector.tensor_add(osb, pso, b_bcast)
                    nc.scalar.dma_start(out_t[:, tt, :], osb)

    nc.compile()
    return nc


def _get_nc():
    if "nc" not in _CACHE:
        _CACHE["nc"] = _build_nc()
    return _CACHE["nc"]


def _make_in_maps(inputs):
    x, W, b, lora_A, lora_B = (
        inputs["x"], inputs["W"], inputs["b"], inputs["lora_A"], inputs["lora_B"]
    )
    x_flat = np.ascontiguousarray(x.reshape(T, D_IN), dtype=np.float32)
    lb = np.ascontiguousarray(lora_B, dtype=np.float32)
    in_maps = []
    for c in range(N_CORES):
        sl = slice(c * O_SHARD, (c + 1) * O_SHARD)
        in_maps.append({
            "x": x_flat,
            "w": np.ascontiguousarray(W[sl], dtype=np.float32),
            "b": np.ascontiguousarray(b[sl].reshape(1, O_SHARD), dtype=np.float32),
            "la": np.ascontiguousarray(lora_A[sl], dtype=np.float32),
            "lb": lb,
        })
    return in_maps


def kernel(x, W, b, lora_A, lora_B):
    from concourse.bass_utils import run_bass_kernel_spmd

    nc = _get_nc()
    in_maps = _make_in_maps(dict(x=x, W=W, b=b, lora_A=lora_A, lora_B=lora_B))
    res = run_bass_kernel_spmd(nc, in_maps, core_ids=list(range(N_CORES)))
    shards = [res.results[c]["out"] for c in range(N_CORES)]
    out = np.concatenate(shards, axis=1).reshape(B_DIM, S_DIM, D_OUT)
    return out.astype(np.float32)



# revision 3
# speedup vs baseline: 1.7425x; 1.7425x over previous
"""LoRA Linear kernel for Trainium2, 8-core hybrid-parallel (4 token groups
x 2 out-feature halves).

out = x @ W^T + b + 2.0 * ((x @ lora_B^T) @ lora_A^T)
    = x @ (W + 2 * lora_A @ lora_B)^T + b

Strategy:
  - Fold the rank-16 LoRA update into the weights ON DEVICE once per
    o-strip (W'^T = W^T + B^T @ (2 A^T), 32 K=16 matmuls per strip),
    reducing the whole problem to a single GEMM + bias.
  - Host marshals x^T and W^T shards pre-tiled in bf16 so every DMA is
    128 partitions x 32KB contiguous and the kernel needs ZERO on-chip
    transposes. All matmuls bf16 (fp32 PSUM accumulate).
  - Sharding 4 token-groups x 2 out-halves minimizes total host->device
    traffic (x replicated 2x, W replicated 4x: ~270MB vs 1.2GB for pure
    tensor-parallel).

Per core: out_shard[2048 t, 2048 o] = x_sh[2048, 4096] @ W'_sh^T + b_sh.
Main loop: 4 o-strips (512) x 4 t-strips (512) x 4 t-tiles (128) x 32 k.
"""

import numpy as np

N_CORES = 8
B_DIM, S_DIM, D_IN, D_OUT = 4, 2048, 4096, 4096
T = B_DIM * S_DIM            # 8192 tokens
TG = 4                       # token groups
OH = 2                       # out-feature halves
T_SH = T // TG               # 2048 tokens per core
O_SH = D_OUT // OH           # 2048 out features per core
R = 16
P = 128
KB = D_IN // P               # 32 k-blocks
NOS = O_SH // 512            # 4 o-strips
NTS = T_SH // 512            # 4 t-strips

_CACHE = {}


def _build_nc():
    import concourse.bacc as bacc
    import concourse.mybir as mybir
    import concourse.tile as tile

    F32 = mybir.dt.float32
    BF16 = mybir.dt.bfloat16

    nc = bacc.Bacc(target_bir_lowering=False)
    # host-tiled layouts (see _make_in_maps):
    #   xt[ts*128+p, kb*512+u] = x_sh[ts*512+u, kb*128+p]   (= x^T tiled)
    #   wt[os*128+p, kb*512+u] = W_sh[os*512+u, kb*128+p]   (= W^T tiled)
    xt_d = nc.dram_tensor("xt", [NTS * P, KB * 512], BF16, kind="ExternalInput")
    wt_d = nc.dram_tensor("wt", [NOS * P, KB * 512], BF16, kind="ExternalInput")
    b_d = nc.dram_tensor("b", [1, O_SH], F32, kind="ExternalInput")
    lat_d = nc.dram_tensor("lat", [R, O_SH], BF16, kind="ExternalInput")  # A^T
    lb_d = nc.dram_tensor("lb", [R, D_IN], BF16, kind="ExternalInput")    # B
    out_d = nc.dram_tensor("out", [T_SH, O_SH], F32, kind="ExternalOutput")

    out_t = out_d[:].rearrange("(tt p) o -> p tt o", p=P)  # [128, 16, 2048]

    with tile.TileContext(nc) as tc:
        with (
            tc.tile_pool(name="const", bufs=1) as const,
            tc.tile_pool(name="xin", bufs=2) as xin,
            tc.tile_pool(name="win", bufs=2) as win,
            tc.tile_pool(name="osb", bufs=3) as osb_pool,
            tc.tile_pool(name="ps_o", bufs=2, space="PSUM") as ps_o,
            tc.tile_pool(name="ps_f", bufs=2, space="PSUM") as ps_f,
        ):
            # resident small tensors
            bsb = const.tile([R, D_IN], BF16)      # lora_B [16, 4096]
            lat2 = const.tile([R, O_SH], BF16)     # 2 * A^T [16, 2048]
            b_bcast = const.tile([P, O_SH], F32)   # bias broadcast over rows
            ones_col = const.tile([1, P], F32)
            b_sb = const.tile([1, O_SH], F32)

            nc.sync.dma_start(bsb, lb_d[:])
            nc.sync.dma_start(lat2, lat_d[:])
            nc.scalar.mul(lat2, lat2, 2.0)
            nc.any.memset(ones_col, 1.0)
            nc.sync.dma_start(b_sb, b_d[:])
            for osi in range(NOS):
                psb = ps_f.tile([P, 512], F32, tag="psf")
                nc.tensor.matmul(
                    psb, ones_col, b_sb[:, osi * 512:(osi + 1) * 512],
                    start=True, stop=True,
                )
                nc.any.tensor_copy(
                    out=b_bcast[:, osi * 512:(osi + 1) * 512], in_=psb
                )

            for osi in range(NOS):
                wsb = win.tile([P, KB, 512], BF16, tag="w")
                nc.sync.dma_start(
                    wsb,
                    wt_d[osi * P:(osi + 1) * P, :].rearrange(
                        "p (kb u) -> p kb u", kb=KB
                    ),
                )
                # fold lora into the strip: W'^T += B^T @ (2 A^T)
                for kb in range(KB):
                    psf = ps_f.tile([P, 512], F32, tag="psf")
                    nc.tensor.matmul(
                        psf,
                        bsb[:, kb * P:(kb + 1) * P],
                        lat2[:, osi * 512:(osi + 1) * 512],
                        start=True, stop=True,
                    )
                    nc.vector.tensor_add(wsb[:, kb, :], psf, wsb[:, kb, :])

                for ts in range(NTS):
                    xsb = xin.tile([P, KB, 512], BF16, tag="x")
                    nc.sync.dma_start(
                        xsb,
                        xt_d[ts * P:(ts + 1) * P, :].rearrange(
                            "p (kb u) -> p kb u", kb=KB
                        ),
                    )
                    for tt in range(4):
                        pso = ps_o.tile([P, 512], F32, tag="pso")
                        for kb in range(KB):
                            nc.tensor.matmul(
                                pso,
                                xsb[:, kb, tt * P:(tt + 1) * P],
                                wsb[:, kb, :],
                                start=(kb == 0),
                                stop=(kb == KB - 1),
                            )
                        osb = osb_pool.tile([P, 512], F32, tag="osb")
                        nc.vector.tensor_add(
                            osb, pso, b_bcast[:, osi * 512:(osi + 1) * 512]
                        )
                        nc.scalar.dma_start(
                            out_t[:, ts * 4 + tt, osi * 512:(osi + 1) * 512], osb
                        )

    nc.compile()
    return nc


def _get_nc():
    if "nc" not in _CACHE:
        _CACHE["nc"] = _build_nc()
    return _CACHE["nc"]


def _make_in_maps(inputs):
    import ml_dtypes

    bf16 = ml_dtypes.bfloat16
    x, W, b, lora_A, lora_B = (
        inputs["x"], inputs["W"], inputs["b"], inputs["lora_A"], inputs["lora_B"]
    )
    x_flat = np.asarray(x, dtype=np.float32).reshape(T, D_IN)
    W = np.asarray(W, dtype=np.float32)
    lb_bf = np.ascontiguousarray(np.asarray(lora_B)).astype(bf16)  # [16, 4096]

    # pre-tile per token-group / o-half (see _build_nc layout comment)
    xts = []
    for tg in range(TG):
        xs = x_flat[tg * T_SH:(tg + 1) * T_SH]           # [2048, 4096]
        h = xs.reshape(NTS, 512, KB, P).transpose(0, 3, 2, 1).astype(bf16)
        xts.append(np.ascontiguousarray(h.reshape(NTS * P, KB * 512)))
    wts, lats, bs = [], [], []
    for oh in range(OH):
        ws = W[oh * O_SH:(oh + 1) * O_SH]                # [2048, 4096]
        h = ws.reshape(NOS, 512, KB, P).transpose(0, 3, 2, 1).astype(bf16)
        wts.append(np.ascontiguousarray(h.reshape(NOS * P, KB * 512)))
        lats.append(np.ascontiguousarray(
            np.asarray(lora_A[oh * O_SH:(oh + 1) * O_SH]).T).astype(bf16))
        bs.append(np.ascontiguousarray(
            np.asarray(b[oh * O_SH:(oh + 1) * O_SH], dtype=np.float32)
        ).reshape(1, O_SH))

    in_maps = []
    for c in range(N_CORES):
        tg, oh = divmod(c, OH)
        in_maps.append({
            "xt": xts[tg],
            "wt": wts[oh],
            "b": bs[oh],
            "lat": lats[oh],
            "lb": lb_bf,
        })
    return in_maps


def kernel(x, W, b, lora_A, lora_B):
    from concourse.bass_utils import run_bass_kernel_spmd

    nc = _get_nc()
    in_maps = _make_in_maps(dict(x=x, W=W, b=b, lora_A=lora_A, lora_B=lora_B))
    res = run_bass_kernel_spmd(nc, in_maps, core_ids=list(range(N_CORES)))
    out = np.empty((T, D_OUT), dtype=np.float32)
    for c in range(N_CORES):
        tg, oh = divmod(c, OH)
        out[tg * T_SH:(tg + 1) * T_SH, oh * O_SH:(oh + 1) * O_SH] = (
            res.results[c]["out"]
        )
    return out.reshape(B_DIM, S_DIM, D_OUT)


# revision 5
# speedup vs baseline: 1.7668x; 1.0139x over previous
"""LoRA Linear kernel for Trainium2, 8-core hybrid-parallel (4 token groups
x 2 out-feature halves).

out = x @ W^T + b + 2.0 * ((x @ lora_B^T) @ lora_A^T)

Per-core strategy (core = token-group tg x out-half oh):
  - Host marshals x^T and W^T shards pre-tiled in bf16 so every DMA is
    128 partitions x 8KB-contiguous and the kernel needs ZERO on-chip
    transposes. All matmuls bf16 (fp32 PSUM accumulate).
  - LoRA: xr^T = lora_B @ x^T computed once per t-strip (32 K=128 MMs)
    during the first o-strip pass; each output psum group then gets one
    extra K=17 matmul [xr^T; ones] @ [2*A^T; b] that adds BOTH the
    rank-16 update and the bias. No DVE work on any matmul's critical
    path.
  - Sharding 4 token-groups x 2 out-halves minimizes host->device
    traffic (~270MB vs 1.2GB for pure tensor-parallel).

Main loop: 4 o-strips (512) x 4 t-strips (512) x 4 t-tiles (128) x 32 k.
Output is written bf16 and upcast to fp32 on the host.
"""

import numpy as np

N_CORES = 8
B_DIM, S_DIM, D_IN, D_OUT = 4, 2048, 4096, 4096
T = B_DIM * S_DIM            # 8192 tokens
TG = 4                       # token groups
OH = 2                       # out-feature halves
T_SH = T // TG               # 2048 tokens per core
O_SH = D_OUT // OH           # 2048 out features per core
R = 16
P = 128
KB = D_IN // P               # 32 k-blocks
NOS = O_SH // 512            # 4 o-strips
NTS = T_SH // 512            # 4 t-strips
NSUB = 4                     # sub-DMAs per strip (8 k-blocks each)
KSUB = KB // NSUB

_CACHE = {}


def _build_nc():
    import concourse.bacc as bacc
    import concourse.mybir as mybir
    import concourse.tile as tile

    F32 = mybir.dt.float32
    BF16 = mybir.dt.bfloat16

    nc = bacc.Bacc(target_bir_lowering=False)
    # host-tiled layouts (see _make_in_maps):
    #   xt[ts*128+p, kb*512+u] = x_sh[ts*512+u, kb*128+p]   (= x^T tiled)
    #   wt[os*128+p, kb*512+u] = W_sh[os*512+u, kb*128+p]   (= W^T tiled)
    #   bt[p, kb*16+r]         = lora_B[r, kb*128+p]        (= B^T tiled)
    #   laug = [2*A_sh^T ; b_sh]  [17, O_SH]
    xt_d = nc.dram_tensor("xt", [NTS * P, KB * 512], BF16, kind="ExternalInput")
    wt_d = nc.dram_tensor("wt", [NOS * P, KB * 512], BF16, kind="ExternalInput")
    bt_d = nc.dram_tensor("bt", [P, KB * R], BF16, kind="ExternalInput")
    laug_d = nc.dram_tensor("laug", [R + 1, O_SH], BF16, kind="ExternalInput")
    out_d = nc.dram_tensor("out", [T_SH, O_SH], BF16, kind="ExternalOutput")

    out_t = out_d[:].rearrange("(tt p) o -> p tt o", p=P)  # [128, 16, 2048]

    with tile.TileContext(nc) as tc:
        with (
            tc.tile_pool(name="const", bufs=1) as const,
            tc.tile_pool(name="xin", bufs=2) as xin,
            tc.tile_pool(name="win", bufs=2) as win,
            tc.tile_pool(name="osb", bufs=3) as osb_pool,
            tc.tile_pool(name="ps_o", bufs=3, space="PSUM") as ps_o,
            tc.tile_pool(name="ps_r", bufs=2, space="PSUM") as ps_r,
        ):
            btT = const.tile([P, KB, R], BF16)      # B^T tiled [128, 32, 16]
            laug = const.tile([R + 1, O_SH], BF16)  # [2*A^T ; b]
            xrT = const.tile([R + 1, T_SH], BF16)   # [xr^T ; ones]

            nc.sync.dma_start(btT, bt_d[:].rearrange("p (kb r) -> p kb r", kb=KB))
            nc.sync.dma_start(laug, laug_d[:])
            # row R stays 1.0 (bias row); rows 0..R-1 are fully overwritten
            # by the psr evictions below. (A partition-16 base slice is not
            # a legal AP, so memset the whole tile.)
            nc.any.memset(xrT, 1.0)

            for osi in range(NOS):
                wsb = win.tile([P, KB, 512], BF16, tag="w")
                for s in range(NSUB):
                    nc.sync.dma_start(
                        wsb[:, s * KSUB:(s + 1) * KSUB, :],
                        wt_d[osi * P:(osi + 1) * P,
                             s * KSUB * 512:(s + 1) * KSUB * 512].rearrange(
                            "p (kb u) -> p kb u", kb=KSUB
                        ),
                    )
                for ts in range(NTS):
                    xsb = xin.tile([P, KB, 512], BF16, tag="x")
                    for s in range(NSUB):
                        nc.sync.dma_start(
                            xsb[:, s * KSUB:(s + 1) * KSUB, :],
                            xt_d[ts * P:(ts + 1) * P,
                                 s * KSUB * 512:(s + 1) * KSUB * 512].rearrange(
                                "p (kb u) -> p kb u", kb=KSUB
                            ),
                        )
                    if osi == 0:
                        # xr^T[r, t-strip] = sum_kb B^T[kb]ᵀ @ x^T[kb]
                        psr = ps_r.tile([R, 512], F32, tag="psr")
                        for kb in range(KB):
                            nc.tensor.matmul(
                                psr,
                                btT[:, kb, :],
                                xsb[:, kb, :],
                                start=(kb == 0),
                                stop=(kb == KB - 1),
                            )
                        nc.vector.tensor_copy(
                            out=xrT[0:R, ts * 512:(ts + 1) * 512], in_=psr
                        )
                    for tt in range(4):
                        pso = ps_o.tile([P, 512], F32, tag="pso")
                        for kb in range(KB):
                            nc.tensor.matmul(
                                pso,
                                xsb[:, kb, tt * P:(tt + 1) * P],
                                wsb[:, kb, :],
                                start=(kb == 0),
                                stop=False,
                            )
                        # rank-16 lora + bias in one K=17 matmul
                        nc.tensor.matmul(
                            pso,
                            xrT[:, ts * 512 + tt * P:ts * 512 + (tt + 1) * P],
                            laug[:, osi * 512:(osi + 1) * 512],
                            start=False,
                            stop=True,
                        )
                        osb = osb_pool.tile([P, 512], BF16, tag="osb")
                        nc.vector.tensor_copy(out=osb, in_=pso)
                        nc.scalar.dma_start(
                            out_t[:, ts * 4 + tt, osi * 512:(osi + 1) * 512], osb
                        )

    nc.compile()
    return nc


def _get_nc():
    if "nc" not in _CACHE:
        _CACHE["nc"] = _build_nc()
    return _CACHE["nc"]


def _make_in_maps(inputs):
    import ml_dtypes

    bf16 = ml_dtypes.bfloat16
    x, W, b, lora_A, lora_B = (
        inputs["x"], inputs["W"], inputs["b"], inputs["lora_A"], inputs["lora_B"]
    )
    x_flat = np.asarray(x, dtype=np.float32).reshape(T, D_IN)
    W = np.asarray(W, dtype=np.float32)
    b = np.asarray(b, dtype=np.float32)
    lora_A = np.asarray(lora_A, dtype=np.float32)
    lora_B = np.asarray(lora_B, dtype=np.float32)

    # B^T tiled: bt[p, kb*16+r] = B[r, kb*128+p]
    bt = np.ascontiguousarray(
        lora_B.T.reshape(KB, P, R).transpose(1, 0, 2).reshape(P, KB * R)
    ).astype(bf16)

    xts = []
    for tg in range(TG):
        xs = x_flat[tg * T_SH:(tg + 1) * T_SH]           # [2048, 4096]
        h = xs.reshape(NTS, 512, KB, P).transpose(0, 3, 2, 1).astype(bf16)
        xts.append(np.ascontiguousarray(h.reshape(NTS * P, KB * 512)))
    wts, laugs = [], []
    for oh in range(OH):
        ws = W[oh * O_SH:(oh + 1) * O_SH]                # [2048, 4096]
        h = ws.reshape(NOS, 512, KB, P).transpose(0, 3, 2, 1).astype(bf16)
        wts.append(np.ascontiguousarray(h.reshape(NOS * P, KB * 512)))
        laug = np.empty((R + 1, O_SH), dtype=np.float32)
        laug[:R] = 2.0 * lora_A[oh * O_SH:(oh + 1) * O_SH].T
        laug[R] = b[oh * O_SH:(oh + 1) * O_SH]
        laugs.append(laug.astype(bf16))

    in_maps = []
    for c in range(N_CORES):
        tg, oh = divmod(c, OH)
        in_maps.append({
            "xt": xts[tg],
            "wt": wts[oh],
            "bt": bt,
            "laug": laugs[oh],
        })
    return in_maps


def kernel(x, W, b, lora_A, lora_B):
    from concourse.bass_utils import run_bass_kernel_spmd

    nc = _get_nc()
    in_maps = _make_in_maps(dict(x=x, W=W, b=b, lora_A=lora_A, lora_B=lora_B))
    res = run_bass_kernel_spmd(nc, in_maps, core_ids=list(range(N_CORES)))
    out = np.empty((T, D_OUT), dtype=np.float32)
    for c in range(N_CORES):
        tg, oh = divmod(c, OH)
        out[tg * T_SH:(tg + 1) * T_SH, oh * O_SH:(oh + 1) * O_SH] = (
            res.results[c]["out"].astype(np.float32)
        )
    return out.reshape(B_DIM, S_DIM, D_OUT)


# revision 7
# speedup vs baseline: 1.7993x; 1.0184x over previous
"""LoRA Linear kernel for Trainium2, 8-core hybrid-parallel (4 token groups
x 2 out-feature halves).

out = x @ W^T + b + 2.0 * ((x @ lora_B^T) @ lora_A^T)

Per-core strategy (core = token-group tg x out-half oh):
  - Host marshals x^T and W^T shards pre-tiled in bf16 so every DMA is
    128 partitions x 8KB-contiguous and the kernel needs ZERO on-chip
    transposes. All matmuls bf16 (fp32 PSUM accumulate).
  - LoRA: xr^T = lora_B @ x^T computed once per t-strip (32 K=128 MMs)
    during the first o-strip pass; each output psum group then gets one
    extra K=17 matmul [xr^T; ones] @ [2*A^T; b] that adds BOTH the
    rank-16 update and the bias. No DVE work on any matmul's critical
    path.
  - Sharding 4 token-groups x 2 out-halves minimizes host->device
    traffic (~270MB vs 1.2GB for pure tensor-parallel).

Main loop: 4 o-strips (512) x 4 t-strips (512) x 4 t-tiles (128) x 32 k.
Output is written bf16 and upcast to fp32 on the host.
"""

import numpy as np

N_CORES = 8
B_DIM, S_DIM, D_IN, D_OUT = 4, 2048, 4096, 4096
T = B_DIM * S_DIM            # 8192 tokens
TG = 4                       # token groups
OH = 2                       # out-feature halves
T_SH = T // TG               # 2048 tokens per core
O_SH = D_OUT // OH           # 2048 out features per core
R = 16
P = 128
KB = D_IN // P               # 32 k-blocks
NOS = O_SH // 512            # 4 o-strips
NTS = T_SH // 512            # 4 t-strips
NSUB = 4                     # sub-DMAs per strip (8 k-blocks each)
KSUB = KB // NSUB

_CACHE = {}


def _build_nc():
    import concourse.bacc as bacc
    import concourse.mybir as mybir
    import concourse.tile as tile

    F32 = mybir.dt.float32
    BF16 = mybir.dt.bfloat16

    nc = bacc.Bacc(target_bir_lowering=False)
    # host-tiled layouts (see _make_in_maps):
    #   xt[ts*128+p, kb*512+u] = x_sh[ts*512+u, kb*128+p]   (= x^T tiled)
    #   wt[os*128+p, kb*512+u] = W_sh[os*512+u, kb*128+p]   (= W^T tiled)
    #   bt[p, kb*16+r]         = lora_B[r, kb*128+p]        (= B^T tiled)
    #   laug = [2*A_sh^T ; b_sh]  [17, O_SH]
    xt_d = nc.dram_tensor("xt", [NTS * P, KB * 512], BF16, kind="ExternalInput")
    wt_d = nc.dram_tensor("wt", [NOS * P, KB * 512], BF16, kind="ExternalInput")
    bt_d = nc.dram_tensor("bt", [P, KB * R], BF16, kind="ExternalInput")
    laug_d = nc.dram_tensor("laug", [R + 1, O_SH], BF16, kind="ExternalInput")
    out_d = nc.dram_tensor("out", [T_SH, O_SH], BF16, kind="ExternalOutput")

    out_t = out_d[:].rearrange("(tt p) o -> p tt o", p=P)  # [128, 16, 2048]

    with tile.TileContext(nc) as tc:
        with (
            tc.tile_pool(name="const", bufs=1) as const,
            tc.tile_pool(name="xin", bufs=2) as xin,
            tc.tile_pool(name="win", bufs=2) as win,
            tc.tile_pool(name="osb", bufs=3) as osb_pool,
            tc.tile_pool(name="ps_o", bufs=4, space="PSUM") as ps_o,
            tc.tile_pool(name="ps_r", bufs=2, space="PSUM") as ps_r,
        ):
            btT = const.tile([P, KB, R], BF16)      # B^T tiled [128, 32, 16]
            laug = const.tile([R + 1, O_SH], BF16)  # [2*A^T ; b]
            xrT = const.tile([R + 1, T_SH], BF16)   # [xr^T ; ones]

            nc.sync.dma_start(btT, bt_d[:].rearrange("p (kb r) -> p kb r", kb=KB))
            nc.sync.dma_start(laug, laug_d[:])
            # row R stays 1.0 (bias row); rows 0..R-1 are fully overwritten
            # by the psr evictions below. (A partition-16 base slice is not
            # a legal AP, so memset the whole tile.)
            nc.any.memset(xrT, 1.0)

            def load_x_strip(ts):
                xsb = xin.tile([P, KB, 512], BF16, tag="x")
                for s in range(NSUB):
                    nc.sync.dma_start(
                        xsb[:, s * KSUB:(s + 1) * KSUB, :],
                        xt_d[ts * P:(ts + 1) * P,
                             s * KSUB * 512:(s + 1) * KSUB * 512].rearrange(
                            "p (kb u) -> p kb u", kb=KSUB
                        ),
                    )
                return xsb

            # first x strip queued ahead of the first W strip: the xr
            # prologue matmuls only need x + btT, so PE starts earlier.
            xsb0 = load_x_strip(0)

            for osi in range(NOS):
                wsb = win.tile([P, KB, 512], BF16, tag="w")
                for s in range(NSUB):
                    nc.sync.dma_start(
                        wsb[:, s * KSUB:(s + 1) * KSUB, :],
                        wt_d[osi * P:(osi + 1) * P,
                             s * KSUB * 512:(s + 1) * KSUB * 512].rearrange(
                            "p (kb u) -> p kb u", kb=KSUB
                        ),
                    )
                for ts in range(NTS):
                    if osi == 0 and ts == 0:
                        xsb = xsb0
                    else:
                        xsb = load_x_strip(ts)
                    if osi == 0:
                        # xr^T[r, t-strip] = sum_kb B^T[kb]ᵀ @ x^T[kb]
                        psr = ps_r.tile([R, 512], F32, tag="psr")
                        for kb in range(KB):
                            nc.tensor.matmul(
                                psr,
                                btT[:, kb, :],
                                xsb[:, kb, :],
                                start=(kb == 0),
                                stop=(kb == KB - 1),
                            )
                        nc.vector.tensor_copy(
                            out=xrT[0:R, ts * 512:(ts + 1) * 512], in_=psr
                        )
                    for tt in range(4):
                        pso = ps_o.tile([P, 512], F32, tag="pso")
                        for kb in range(KB):
                            nc.tensor.matmul(
                                pso,
                                xsb[:, kb, tt * P:(tt + 1) * P],
                                wsb[:, kb, :],
                                start=(kb == 0),
                                stop=False,
                            )
                        # rank-16 lora + bias in one K=17 matmul
                        nc.tensor.matmul(
                            pso,
                            xrT[:, ts * 512 + tt * P:ts * 512 + (tt + 1) * P],
                            laug[:, osi * 512:(osi + 1) * 512],
                            start=False,
                            stop=True,
                        )
                        osb = osb_pool.tile([P, 512], BF16, tag="osb")
                        nc.vector.tensor_copy(out=osb, in_=pso)
                        nc.scalar.dma_start(
                            out_t[:, ts * 4 + tt, osi * 512:(osi + 1) * 512], osb
                        )

    nc.compile()
    return nc


def _get_nc():
    if "nc" not in _CACHE:
        _CACHE["nc"] = _build_nc()
    return _CACHE["nc"]


def _make_in_maps(inputs):
    import ml_dtypes

    bf16 = ml_dtypes.bfloat16
    x, W, b, lora_A, lora_B = (
        inputs["x"], inputs["W"], inputs["b"], inputs["lora_A"], inputs["lora_B"]
    )
    x_flat = np.asarray(x, dtype=np.float32).reshape(T, D_IN)
    W = np.asarray(W, dtype=np.float32)
    b = np.asarray(b, dtype=np.float32)
    lora_A = np.asarray(lora_A, dtype=np.float32)
    lora_B = np.asarray(lora_B, dtype=np.float32)

    # B^T tiled: bt[p, kb*16+r] = B[r, kb*128+p]
    bt = np.ascontiguousarray(
        lora_B.T.reshape(KB, P, R).transpose(1, 0, 2).reshape(P, KB * R)
    ).astype(bf16)

    xts = []
    for tg in range(TG):
        xs = x_flat[tg * T_SH:(tg + 1) * T_SH]           # [2048, 4096]
        h = xs.reshape(NTS, 512, KB, P).transpose(0, 3, 2, 1).astype(bf16)
        xts.append(np.ascontiguousarray(h.reshape(NTS * P, KB * 512)))
    wts, laugs = [], []
    for oh in range(OH):
        ws = W[oh * O_SH:(oh + 1) * O_SH]                # [2048, 4096]
        h = ws.reshape(NOS, 512, KB, P).transpose(0, 3, 2, 1).astype(bf16)
        wts.append(np.ascontiguousarray(h.reshape(NOS * P, KB * 512)))
        laug = np.empty((R + 1, O_SH), dtype=np.float32)
        laug[:R] = 2.0 * lora_A[oh * O_SH:(oh + 1) * O_SH].T
        laug[R] = b[oh * O_SH:(oh + 1) * O_SH]
        laugs.append(laug.astype(bf16))

    in_maps = []
    for c in range(N_CORES):
        tg, oh = divmod(c, OH)
        in_maps.append({
            "xt": xts[tg],
            "wt": wts[oh],
            "bt": bt,
            "laug": laugs[oh],
        })
    return in_maps


def kernel(x, W, b, lora_A, lora_B):
    from concourse.bass_utils import run_bass_kernel_spmd

    nc = _get_nc()
    in_maps = _make_in_maps(dict(x=x, W=W, b=b, lora_A=lora_A, lora_B=lora_B))
    res = run_bass_kernel_spmd(nc, in_maps, core_ids=list(range(N_CORES)))
    out = np.empty((T, D_OUT), dtype=np.float32)
    for c in range(N_CORES):
        tg, oh = divmod(c, OH)
        out[tg * T_SH:(tg + 1) * T_SH, oh * O_SH:(oh + 1) * O_SH] = (
            res.results[c]["out"].astype(np.float32)
        )
    return out.reshape(B_DIM, S_DIM, D_OUT)


# revision 15
# speedup vs baseline: 1.8326x; 1.0185x over previous
"""LoRA Linear kernel for Trainium2, 8-core hybrid-parallel (4 token groups
x 2 out-feature halves).

out = x @ W^T + b + 2.0 * ((x @ lora_B^T) @ lora_A^T)

Per-core strategy (core = token-group tg x out-half oh):
  - Host marshals x^T and W^T shards pre-tiled in bf16 so every DMA is
    128 partitions x 8KB-contiguous and the kernel needs ZERO on-chip
    transposes. All matmuls bf16 (fp32 PSUM accumulate).
  - LoRA: xr^T = lora_B @ x^T computed once per t-strip (32 K=128 MMs)
    during the first o-strip pass; each output psum group then gets one
    extra K=17 matmul [xr^T; ones] @ [2*A^T; b] that adds BOTH the
    rank-16 update and the bias. No DVE work on any matmul's critical
    path.
  - Sharding 4 token-groups x 2 out-halves minimizes host->device
    traffic (~270MB vs 1.2GB for pure tensor-parallel).

Main loop: 4 o-strips (512) x 4 t-strips (512) x 4 t-tiles (128) x 32 k.
Output is written bf16 and upcast to fp32 on the host.
"""

import numpy as np

N_CORES = 8
B_DIM, S_DIM, D_IN, D_OUT = 4, 2048, 4096, 4096
T = B_DIM * S_DIM            # 8192 tokens
TG = 4                       # token groups
OH = 2                       # out-feature halves
T_SH = T // TG               # 2048 tokens per core
O_SH = D_OUT // OH           # 2048 out features per core
R = 16
P = 128
KB = D_IN // P               # 32 k-blocks
NOS = O_SH // 512            # 4 o-strips
NTS = T_SH // 512            # 4 t-strips
NSUB = 4                     # sub-DMAs per strip (8 k-blocks each)
KSUB = KB // NSUB

_CACHE = {}


def _build_nc():
    import concourse.bacc as bacc
    import concourse.mybir as mybir
    import concourse.tile as tile

    F32 = mybir.dt.float32
    BF16 = mybir.dt.bfloat16

    nc = bacc.Bacc(target_bir_lowering=False)
    # host-tiled layouts (see _make_in_maps):
    #   xt[ts*128+p, kb*512+u] = x_sh[ts*512+u, kb*128+p]   (= x^T tiled)
    #   wt[os*128+p, kb*512+u] = W_sh[os*512+u, kb*128+p]   (= W^T tiled)
    #   bt[p, kb*16+r]         = lora_B[r, kb*128+p]        (= B^T tiled)
    #   laug = [2*A_sh^T ; b_sh]  [17, O_SH]
    xt_d = nc.dram_tensor("xt", [NTS * P, KB * 512], BF16, kind="ExternalInput")
    wt_d = nc.dram_tensor("wt", [NOS * P, KB * 512], BF16, kind="ExternalInput")
    # bt / laug are zero-padded to full 128-partition operands so the xr and
    # lora matmuls are full-array ops (partial row/col-group matmuls stall
    # the LDWEIGHTS pull-ahead next to full-array matmuls).
    bt_d = nc.dram_tensor("bt", [P, KB * P], BF16, kind="ExternalInput")
    laug_d = nc.dram_tensor("laug", [P, O_SH], BF16, kind="ExternalInput")
    out_d = nc.dram_tensor("out", [T_SH, O_SH], BF16, kind="ExternalOutput")

    out_t = out_d[:].rearrange("(tt p) o -> p tt o", p=P)  # [128, 16, 2048]

    with tile.TileContext(nc) as tc:
        with (
            tc.tile_pool(name="const", bufs=1) as const,
            tc.tile_pool(name="xin", bufs=3) as xin,
            tc.tile_pool(name="win", bufs=2) as win,
            tc.tile_pool(name="osb", bufs=3) as osb_pool,
            tc.tile_pool(name="ps_o", bufs=4, space="PSUM") as ps_o,
            tc.tile_pool(name="ps_r", bufs=2, space="PSUM") as ps_r,
        ):
            btT = const.tile([P, KB, P], BF16)   # B^T tiled, cols 16+ zero
            laug = const.tile([P, O_SH], BF16)   # rows: 2*A^T(0-15), b(32), 0
            xrT = const.tile([P, T_SH], BF16)    # rows: xr^T(0-15), ones(32), 0

            nc.sync.dma_start(btT, bt_d[:].rearrange("p (kb r) -> p kb r", kb=KB))
            nc.sync.dma_start(laug, laug_d[:])
            # rows 0-15 get the xr evictions; row 32 is the bias-ones row
            # (32 is a legal partition base; 16 is not); the rest stay 0.
            nc.any.memset(xrT, 0.0)
            nc.any.memset(xrT[32:33, :], 1.0)

            def load_x_strip(ts):
                xsb = xin.tile([P, KB, 512], BF16, tag="x")
                for s in range(NSUB):
                    nc.sync.dma_start(
                        xsb[:, s * KSUB:(s + 1) * KSUB, :],
                        xt_d[ts * P:(ts + 1) * P,
                             s * KSUB * 512:(s + 1) * KSUB * 512].rearrange(
                            "p (kb u) -> p kb u", kb=KSUB
                        ),
                    )
                return xsb

            # first x strip queued ahead of the first W strip: the xr
            # prologue matmuls only need x + btT, so PE starts earlier.
            xsb0 = load_x_strip(0)

            for osi in range(NOS):
                wsb = win.tile([P, KB, 512], BF16, tag="w")
                for s in range(NSUB):
                    nc.sync.dma_start(
                        wsb[:, s * KSUB:(s + 1) * KSUB, :],
                        wt_d[osi * P:(osi + 1) * P,
                             s * KSUB * 512:(s + 1) * KSUB * 512].rearrange(
                            "p (kb u) -> p kb u", kb=KSUB
                        ),
                    )
                for ts in range(NTS):
                    if osi == 0 and ts == 0:
                        xsb = xsb0
                    else:
                        xsb = load_x_strip(ts)
                    if osi == 0:
                        # xr^T[r, t-strip] = sum_kb B^T[kb]ᵀ @ x^T[kb]
                        psr = ps_r.tile([P, 512], F32, tag="psr")
                        for kb in range(KB):
                            nc.tensor.matmul(
                                psr,
                                btT[:, kb, :],
                                xsb[:, kb, :],
                                start=(kb == 0),
                                stop=(kb == KB - 1),
                            )
                        nc.vector.tensor_copy(
                            out=xrT[0:R, ts * 512:(ts + 1) * 512],
                            in_=psr[0:R, :],
                        )
                    for tt in range(4):
                        pso = ps_o.tile([P, 512], F32, tag="pso")
                        for kb in range(KB):
                            nc.tensor.matmul(
                                pso,
                                xsb[:, kb, tt * P:(tt + 1) * P],
                                wsb[:, kb, :],
                                start=(kb == 0),
                                stop=False,
                            )
                        # rank-16 lora + bias in one full-array matmul
                        # (zero-padded K: rows 0-15 xr, row 32 ones/bias)
                        nc.tensor.matmul(
                            pso,
                            xrT[:, ts * 512 + tt * P:ts * 512 + (tt + 1) * P],
                            laug[:, osi * 512:(osi + 1) * 512],
                            start=False,
                            stop=True,
                        )
                        osb = osb_pool.tile([P, 512], BF16, tag="osb")
                        nc.vector.tensor_copy(out=osb, in_=pso)
                        nc.scalar.dma_start(
                            out_t[:, ts * 4 + tt, osi * 512:(osi + 1) * 512], osb
                        )

    nc.compile()
    return nc


def _get_nc():
    if "nc" not in _CACHE:
        _CACHE["nc"] = _build_nc()
    return _CACHE["nc"]


def _make_in_maps(inputs):
    import ml_dtypes

    bf16 = ml_dtypes.bfloat16
    x, W, b, lora_A, lora_B = (
        inputs["x"], inputs["W"], inputs["b"], inputs["lora_A"], inputs["lora_B"]
    )
    x_flat = np.asarray(x, dtype=np.float32).reshape(T, D_IN)
    W = np.asarray(W, dtype=np.float32)
    b = np.asarray(b, dtype=np.float32)
    lora_A = np.asarray(lora_A, dtype=np.float32)
    lora_B = np.asarray(lora_B, dtype=np.float32)

    # B^T tiled and zero-padded: bt[p, kb*128+r] = B[r, kb*128+p] for r<16
    bt = np.zeros((P, KB, P), dtype=np.float32)
    bt[:, :, :R] = lora_B.T.reshape(KB, P, R).transpose(1, 0, 2)
    bt = bt.reshape(P, KB * P).astype(bf16)

    xts = []
    for tg in range(TG):
        xs = x_flat[tg * T_SH:(tg + 1) * T_SH]           # [2048, 4096]
        h = xs.reshape(NTS, 512, KB, P).transpose(0, 3, 2, 1).astype(bf16)
        xts.append(np.ascontiguousarray(h.reshape(NTS * P, KB * 512)))
    wts, laugs = [], []
    for oh in range(OH):
        ws = W[oh * O_SH:(oh + 1) * O_SH]                # [2048, 4096]
        h = ws.reshape(NOS, 512, KB, P).transpose(0, 3, 2, 1).astype(bf16)
        wts.append(np.ascontiguousarray(h.reshape(NOS * P, KB * 512)))
        laug = np.zeros((P, O_SH), dtype=np.float32)
        laug[:R] = 2.0 * lora_A[oh * O_SH:(oh + 1) * O_SH].T
        laug[32] = b[oh * O_SH:(oh + 1) * O_SH]
        laugs.append(laug.astype(bf16))

    in_maps = []
    for c in range(N_CORES):
        tg, oh = divmod(c, OH)
        in_maps.append({
            "xt": xts[tg],
            "wt": wts[oh],
            "bt": bt,
            "laug": laugs[oh],
        })
    return in_maps


def kernel(x, W, b, lora_A, lora_B):
    from concourse.bass_utils import run_bass_kernel_spmd

    nc = _get_nc()
    in_maps = _make_in_maps(dict(x=x, W=W, b=b, lora_A=lora_A, lora_B=lora_B))
    res = run_bass_kernel_spmd(nc, in_maps, core_ids=list(range(N_CORES)))
    out = np.empty((T, D_OUT), dtype=np.float32)
    for c in range(N_CORES):
        tg, oh = divmod(c, OH)
        out[tg * T_SH:(tg + 1) * T_SH, oh * O_SH:(oh + 1) * O_SH] = (
            res.results[c]["out"].astype(np.float32)
        )
    return out.reshape(B_DIM, S_DIM, D_OUT)


# revision 16
# speedup vs baseline: 1.8572x; 1.0134x over previous
"""LoRA Linear kernel for Trainium2, 8-core hybrid-parallel (4 token groups
x 2 out-feature halves).

out = x @ W^T + b + 2.0 * ((x @ lora_B^T) @ lora_A^T)

Per-core strategy (core = token-group tg x out-half oh):
  - Host marshals x^T and W^T shards pre-tiled in bf16 so every DMA is
    128 partitions x 8KB-contiguous and the kernel needs ZERO on-chip
    transposes. All matmuls bf16 (fp32 PSUM accumulate).
  - LoRA: xr^T = lora_B @ x^T computed once per t-strip (32 K=128 MMs)
    during the first o-strip pass; each output psum group then gets one
    extra K=17 matmul [xr^T; ones] @ [2*A^T; b] that adds BOTH the
    rank-16 update and the bias. No DVE work on any matmul's critical
    path.
  - Sharding 4 token-groups x 2 out-halves minimizes host->device
    traffic (~270MB vs 1.2GB for pure tensor-parallel).

Main loop: 4 o-strips (512) x 4 t-strips (512) x 4 t-tiles (128) x 32 k.
Output is written bf16 and upcast to fp32 on the host.
"""

import numpy as np

N_CORES = 8
B_DIM, S_DIM, D_IN, D_OUT = 4, 2048, 4096, 4096
T = B_DIM * S_DIM            # 8192 tokens
TG = 4                       # token groups
OH = 2                       # out-feature halves
T_SH = T // TG               # 2048 tokens per core
O_SH = D_OUT // OH           # 2048 out features per core
R = 16
P = 128
KB = D_IN // P               # 32 k-blocks
NOS = O_SH // 512            # 4 o-strips
NTS = T_SH // 512            # 4 t-strips
NSUB = 4                     # sub-DMAs per strip (8 k-blocks each)
KSUB = KB // NSUB

_CACHE = {}


def _build_nc():
    import concourse.bacc as bacc
    import concourse.mybir as mybir
    import concourse.tile as tile

    F32 = mybir.dt.float32
    BF16 = mybir.dt.bfloat16

    nc = bacc.Bacc(target_bir_lowering=False)
    # host-tiled layouts (see _make_in_maps):
    #   xt[ts*128+p, kb*512+u] = x_sh[ts*512+u, kb*128+p]   (= x^T tiled)
    #   wt[os*128+p, kb*512+u] = W_sh[os*512+u, kb*128+p]   (= W^T tiled)
    #   bt[p, kb*16+r]         = lora_B[r, kb*128+p]        (= B^T tiled)
    #   laug = [2*A_sh^T ; b_sh]  [17, O_SH]
    xt_d = nc.dram_tensor("xt", [NTS * P, KB * 512], BF16, kind="ExternalInput")
    wt_d = nc.dram_tensor("wt", [NOS * P, KB * 512], BF16, kind="ExternalInput")
    # bt / laug are zero-padded to full 128-partition operands so the xr and
    # lora matmuls are full-array ops (partial row/col-group matmuls stall
    # the LDWEIGHTS pull-ahead next to full-array matmuls).
    bt_d = nc.dram_tensor("bt", [P, KB * P], BF16, kind="ExternalInput")
    laug_d = nc.dram_tensor("laug", [P, O_SH], BF16, kind="ExternalInput")
    out_d = nc.dram_tensor("out", [T_SH, O_SH], BF16, kind="ExternalOutput")

    out_t = out_d[:].rearrange("(tt p) o -> p tt o", p=P)  # [128, 16, 2048]

    with tile.TileContext(nc) as tc:
        with (
            tc.tile_pool(name="const", bufs=1) as const,
            tc.tile_pool(name="xin", bufs=3) as xin,
            tc.tile_pool(name="win", bufs=2) as win,
            tc.tile_pool(name="osb", bufs=3) as osb_pool,
            tc.tile_pool(name="ps_o", bufs=4, space="PSUM") as ps_o,
            tc.tile_pool(name="ps_r", bufs=2, space="PSUM") as ps_r,
        ):
            btT = const.tile([P, KB, P], BF16)   # B^T tiled, cols 16+ zero
            laug = const.tile([P, O_SH], BF16)   # rows: 2*A^T(0-15), b(32), 0
            xrT = const.tile([P, T_SH], BF16)    # rows: xr^T(0-15), ones(32), 0

            # rows 0-15 get the xr evictions; row 32 is the bias-ones row
            # (32 is a legal partition base; 16 is not); the rest stay 0.
            nc.any.memset(xrT, 0.0)
            nc.any.memset(xrT[32:33, :], 1.0)
            nc.sync.dma_start(btT, bt_d[:].rearrange("p (kb r) -> p kb r", kb=KB))

            def x_sub(xsb, ts, s):
                nc.sync.dma_start(
                    xsb[:, s * KSUB:(s + 1) * KSUB, :],
                    xt_d[ts * P:(ts + 1) * P,
                         s * KSUB * 512:(s + 1) * KSUB * 512].rearrange(
                        "p (kb u) -> p kb u", kb=KSUB
                    ),
                )

            def w_sub(wsb, osi, s):
                nc.sync.dma_start(
                    wsb[:, s * KSUB:(s + 1) * KSUB, :],
                    wt_d[osi * P:(osi + 1) * P,
                         s * KSUB * 512:(s + 1) * KSUB * 512].rearrange(
                        "p (kb u) -> p kb u", kb=KSUB
                    ),
                )

            # startup: interleave the first x strip and first W strip so
            # the xr prologue (needs x+btT) and the first main groups
            # (need x+W) both start as soon as their sub-strips land.
            xsb0 = xin.tile([P, KB, 512], BF16, tag="x")
            wsb0 = win.tile([P, KB, 512], BF16, tag="w")
            for s in range(NSUB):
                x_sub(xsb0, 0, s)
                w_sub(wsb0, 0, s)
            nc.sync.dma_start(laug, laug_d[:])

            for osi in range(NOS):
                if osi == 0:
                    wsb = wsb0
                else:
                    wsb = win.tile([P, KB, 512], BF16, tag="w")
                    for s in range(NSUB):
                        w_sub(wsb, osi, s)
                for ts in range(NTS):
                    if osi == 0 and ts == 0:
                        xsb = xsb0
                    else:
                        xsb = xin.tile([P, KB, 512], BF16, tag="x")
                        for s in range(NSUB):
                            x_sub(xsb, ts, s)
                    if osi == 0:
                        # xr^T[r, t-strip] = sum_kb B^T[kb]ᵀ @ x^T[kb]
                        psr = ps_r.tile([P, 512], F32, tag="psr")
                        for kb in range(KB):
                            nc.tensor.matmul(
                                psr,
                                btT[:, kb, :],
                                xsb[:, kb, :],
                                start=(kb == 0),
                                stop=(kb == KB - 1),
                            )
                        nc.vector.tensor_copy(
                            out=xrT[0:R, ts * 512:(ts + 1) * 512],
                            in_=psr[0:R, :],
                        )
                    for tt in range(4):
                        pso = ps_o.tile([P, 512], F32, tag="pso")
                        for kb in range(KB):
                            nc.tensor.matmul(
                                pso,
                                xsb[:, kb, tt * P:(tt + 1) * P],
                                wsb[:, kb, :],
                                start=(kb == 0),
                                stop=False,
                            )
                        # rank-16 lora + bias in one full-array matmul
                        # (zero-padded K: rows 0-15 xr, row 32 ones/bias)
                        nc.tensor.matmul(
                            pso,
                            xrT[:, ts * 512 + tt * P:ts * 512 + (tt + 1) * P],
                            laug[:, osi * 512:(osi + 1) * 512],
                            start=False,
                            stop=True,
                        )
                        osb = osb_pool.tile([P, 512], BF16, tag="osb")
                        nc.vector.tensor_copy(out=osb, in_=pso)
                        nc.scalar.dma_start(
                            out_t[:, ts * 4 + tt, osi * 512:(osi + 1) * 512], osb
                        )

    nc.compile()
    return nc


def _get_nc():
    if "nc" not in _CACHE:
        _CACHE["nc"] = _build_nc()
    return _CACHE["nc"]


def _make_in_maps(inputs):
    import ml_dtypes

    bf16 = ml_dtypes.bfloat16
    x, W, b, lora_A, lora_B = (
        inputs["x"], inputs["W"], inputs["b"], inputs["lora_A"], inputs["lora_B"]
    )
    x_flat = np.asarray(x, dtype=np.float32).reshape(T, D_IN)
    W = np.asarray(W, dtype=np.float32)
    b = np.asarray(b, dtype=np.float32)
    lora_A = np.asarray(lora_A, dtype=np.float32)
    lora_B = np.asarray(lora_B, dtype=np.float32)

    # B^T tiled and zero-padded: bt[p, kb*128+r] = B[r, kb*128+p] for r<16
    bt = np.zeros((P, KB, P), dtype=np.float32)
    bt[:, :, :R] = lora_B.T.reshape(KB, P, R).transpose(1, 0, 2)
    bt = bt.reshape(P, KB * P).astype(bf16)

    xts = []
    for tg in range(TG):
        xs = x_flat[tg * T_SH:(tg + 1) * T_SH]           # [2048, 4096]
        h = xs.reshape(NTS, 512, KB, P).transpose(0, 3, 2, 1).astype(bf16)
        xts.append(np.ascontiguousarray(h.reshape(NTS * P, KB * 512)))
    wts, laugs = [], []
    for oh in range(OH):
        ws = W[oh * O_SH:(oh + 1) * O_SH]                # [2048, 4096]
        h = ws.reshape(NOS, 512, KB, P).transpose(0, 3, 2, 1).astype(bf16)
        wts.append(np.ascontiguousarray(h.reshape(NOS * P, KB * 512)))
        laug = np.zeros((P, O_SH), dtype=np.float32)
        laug[:R] = 2.0 * lora_A[oh * O_SH:(oh + 1) * O_SH].T
        laug[32] = b[oh * O_SH:(oh + 1) * O_SH]
        laugs.append(laug.astype(bf16))

    in_maps = []
    for c in range(N_CORES):
        tg, oh = divmod(c, OH)
        in_maps.append({
            "xt": xts[tg],
            "wt": wts[oh],
            "bt": bt,
            "laug": laugs[oh],
        })
    return in_maps


def kernel(x, W, b, lora_A, lora_B):
    from concourse.bass_utils import run_bass_kernel_spmd

    nc = _get_nc()
    in_maps = _make_in_maps(dict(x=x, W=W, b=b, lora_A=lora_A, lora_B=lora_B))
    res = run_bass_kernel_spmd(nc, in_maps, core_ids=list(range(N_CORES)))
    out = np.empty((T, D_OUT), dtype=np.float32)
    for c in range(N_CORES):
        tg, oh = divmod(c, OH)
        out[tg * T_SH:(tg + 1) * T_SH, oh * O_SH:(oh + 1) * O_SH] = (
            res.results[c]["out"].astype(np.float32)
        )
    return out.reshape(B_DIM, S_DIM, D_OUT)


# revision 23
# speedup vs baseline: 1.9213x; 1.0346x over previous
"""LoRA Linear kernel for Trainium2, 8-core hybrid-parallel (4 token groups
x 2 out-feature halves).

out = x @ W^T + b + 2.0 * ((x @ lora_B^T) @ lora_A^T)

Per-core strategy (core = token-group tg x out-half oh):
  - Host marshals x^T and W^T shards pre-tiled in bf16 so every DMA is
    128 partitions x 8KB-contiguous and the kernel needs ZERO on-chip
    transposes. All matmuls bf16 (fp32 PSUM accumulate).
  - LoRA: xr^T = lora_B @ x^T computed once per t-strip (32 K=128 MMs)
    during the first o-strip pass; each output psum group then gets one
    extra K=17 matmul [xr^T; ones] @ [2*A^T; b] that adds BOTH the
    rank-16 update and the bias. No DVE work on any matmul's critical
    path.
  - Sharding 4 token-groups x 2 out-halves minimizes host->device
    traffic (~270MB vs 1.2GB for pure tensor-parallel).

Main loop: 4 o-strips (512) x 4 t-strips (512) x 4 t-tiles (128) x 32 k.
Output is written bf16 and upcast to fp32 on the host.
"""

import numpy as np

N_CORES = 8
B_DIM, S_DIM, D_IN, D_OUT = 4, 2048, 4096, 4096
T = B_DIM * S_DIM            # 8192 tokens
TG = 4                       # token groups
OH = 2                       # out-feature halves
T_SH = T // TG               # 2048 tokens per core
O_SH = D_OUT // OH           # 2048 out features per core
R = 16
P = 128
KB = D_IN // P               # 32 k-blocks
NOS = O_SH // 512            # 4 o-strips
NTS = T_SH // 512            # 4 t-strips
NSUB = 4                     # sub-DMAs per strip (8 k-blocks each)
KSUB = KB // NSUB

_CACHE = {}


def _build_nc():
    import concourse.bacc as bacc
    import concourse.mybir as mybir
    import concourse.tile as tile

    F32 = mybir.dt.float32
    BF16 = mybir.dt.bfloat16

    nc = bacc.Bacc(target_bir_lowering=False)
    # host-tiled layouts (see _make_in_maps):
    #   xt[ts*128+p, kb*512+u] = x_sh[ts*512+u, kb*128+p]   (= x^T tiled)
    #   wt[os*128+p, kb*512+u] = W_sh[os*512+u, kb*128+p]   (= W^T tiled)
    #   bt[p, kb*16+r]         = lora_B[r, kb*128+p]        (= B^T tiled)
    #   laug = [2*A_sh^T ; b_sh]  [17, O_SH]
    xt_d = nc.dram_tensor("xt", [NTS * P, KB * 512], BF16, kind="ExternalInput")
    wt_d = nc.dram_tensor("wt", [NOS * P, KB * 512], BF16, kind="ExternalInput")
    # laug is zero-padded to a full 128-partition operand: rows 32-47 /
    # 64-79 / 96-111 hold copies of 2*A^T (one per xr partial group), row 0
    # holds b. The lora matmul contracts all 128 rows, summing the three
    # xr partials and the bias in one shot.
    bt_d = nc.dram_tensor("bt", [P, KB * R], BF16, kind="ExternalInput")
    laug_d = nc.dram_tensor("laug", [P, O_SH], BF16, kind="ExternalInput")
    out_d = nc.dram_tensor("out", [T_SH, O_SH], BF16, kind="ExternalOutput")

    out_t = out_d[:].rearrange("(tt p) o -> p tt o", p=P)  # [128, 16, 2048]

    with tile.TileContext(nc) as tc:
        with (
            tc.tile_pool(name="const", bufs=1) as const,
            tc.tile_pool(name="xin", bufs=3) as xin,
            tc.tile_pool(name="win", bufs=2) as win,
            tc.tile_pool(name="osb", bufs=3) as osb_pool,
            tc.tile_pool(name="ps_o", bufs=4, space="PSUM") as ps_o,
            tc.tile_pool(name="ps_r", bufs=2, space="PSUM") as ps_r,
        ):
            btT = const.tile([P, KB, R], BF16)   # B^T tiled [128, 32, 16]
            laug = const.tile([P, O_SH], BF16)   # 2*A^T at rows 32/64/96+, b at 0
            xrT = const.tile([P, T_SH], BF16)    # xr partials at 32/64/96+, ones at 0

            # rows 32-47 / 64-79 / 96-111 get the three packed-xr partial
            # evictions; row 0 is the bias-ones row; the rest stay 0.
            nc.any.memset(xrT, 0.0)
            nc.any.memset(xrT[0:1, :], 1.0)
            nc.sync.dma_start(btT, bt_d[:].rearrange("p (kb r) -> p kb r", kb=KB))

            def x_sub(xsb, ts, s):
                nc.sync.dma_start(
                    xsb[:, s * KSUB:(s + 1) * KSUB, :],
                    xt_d[ts * P:(ts + 1) * P,
                         s * KSUB * 512:(s + 1) * KSUB * 512].rearrange(
                        "p (kb u) -> p kb u", kb=KSUB
                    ),
                )

            def w_sub(wsb, osi, s):
                nc.sync.dma_start(
                    wsb[:, s * KSUB:(s + 1) * KSUB, :],
                    wt_d[osi * P:(osi + 1) * P,
                         s * KSUB * 512:(s + 1) * KSUB * 512].rearrange(
                        "p (kb u) -> p kb u", kb=KSUB
                    ),
                )

            # startup: interleave the first x strip and first W strip so
            # the xr prologue (needs x+btT) and the first main groups
            # (need x+W) both start as soon as their sub-strips land.
            xsb0 = xin.tile([P, KB, 512], BF16, tag="x")
            wsb0 = win.tile([P, KB, 512], BF16, tag="w")
            for s in range(NSUB):
                x_sub(xsb0, 0, s)
                w_sub(wsb0, 0, s)
            nc.sync.dma_start(laug, laug_d[:])

            for osi in range(NOS):
                if osi == 0:
                    wsb = wsb0
                else:
                    wsb = win.tile([P, KB, 512], BF16, tag="w")
                    for s in range(NSUB):
                        w_sub(wsb, osi, s)
                for ts in range(NTS):
                    if osi == 0 and ts == 0:
                        xsb = xsb0
                    else:
                        xsb = xin.tile([P, KB, 512], BF16, tag="x")
                        for s in range(NSUB):
                            x_sub(xsb, ts, s)
                    if osi == 0:
                        # xr^T = B @ x^T, col-tiled 3x concurrent: partial
                        # sums over kb-thirds land at psum partition groups
                        # 32/64/96; the lora matmul's replicated 2*A^T rows
                        # absorb the cross-group reduction for free.
                        psr = ps_r.tile([P, 512], F32, tag="psr")
                        splits = [(0, 11, 32), (11, 22, 64), (22, KB, 96)]
                        for q in range(11):
                            for lo, hi, base in splits:
                                kb = lo + q
                                if kb >= hi:
                                    continue
                                nc.tensor.matmul(
                                    psr[base:base + R, :],
                                    btT[:, kb, :],
                                    xsb[:, kb, :],
                                    start=(kb == lo),
                                    stop=(kb == hi - 1),
                                    tile_position=(0, base),
                                )
                        for _, _, base in splits:
                            nc.vector.tensor_copy(
                                out=xrT[base:base + R, ts * 512:(ts + 1) * 512],
                                in_=psr[base:base + R, :],
                            )
                    for tt in range(4):
                        pso = ps_o.tile([P, 512], F32, tag="pso")
                        for kb in range(KB):
                            nc.tensor.matmul(
                                pso,
                                xsb[:, kb, tt * P:(tt + 1) * P],
                                wsb[:, kb, :],
                                start=(kb == 0),
                                stop=False,
                            )
                        # rank-16 lora + bias in one full-array matmul
                        # (zero-padded K: rows 0-15 xr, row 32 ones/bias)
                        nc.tensor.matmul(
                            pso,
                            xrT[:, ts * 512 + tt * P:ts * 512 + (tt + 1) * P],
                            laug[:, osi * 512:(osi + 1) * 512],
                            start=False,
                            stop=True,
                        )
                        osb = osb_pool.tile([P, 512], BF16, tag="osb")
                        nc.vector.tensor_copy(out=osb, in_=pso)
                        nc.scalar.dma_start(
                            out_t[:, ts * 4 + tt, osi * 512:(osi + 1) * 512], osb
                        )

    nc.compile()
    return nc


def _get_nc():
    if "nc" not in _CACHE:
        _CACHE["nc"] = _build_nc()
    return _CACHE["nc"]


def _make_in_maps(inputs):
    import ml_dtypes

    bf16 = ml_dtypes.bfloat16
    x, W, b, lora_A, lora_B = (
        inputs["x"], inputs["W"], inputs["b"], inputs["lora_A"], inputs["lora_B"]
    )
    x_flat = np.asarray(x, dtype=np.float32).reshape(T, D_IN)
    W = np.asarray(W, dtype=np.float32)
    b = np.asarray(b, dtype=np.float32)
    lora_A = np.asarray(lora_A, dtype=np.float32)
    lora_B = np.asarray(lora_B, dtype=np.float32)

    # B^T tiled: bt[p, kb*16+r] = B[r, kb*128+p]
    bt = np.ascontiguousarray(
        lora_B.T.reshape(KB, P, R).transpose(1, 0, 2).reshape(P, KB * R)
    ).astype(bf16)

    xts = []
    for tg in range(TG):
        xs = x_flat[tg * T_SH:(tg + 1) * T_SH]           # [2048, 4096]
        h = xs.reshape(NTS, 512, KB, P).transpose(0, 3, 2, 1).astype(bf16)
        xts.append(np.ascontiguousarray(h.reshape(NTS * P, KB * 512)))
    wts, laugs = [], []
    for oh in range(OH):
        ws = W[oh * O_SH:(oh + 1) * O_SH]                # [2048, 4096]
        h = ws.reshape(NOS, 512, KB, P).transpose(0, 3, 2, 1).astype(bf16)
        wts.append(np.ascontiguousarray(h.reshape(NOS * P, KB * 512)))
        laug = np.zeros((P, O_SH), dtype=np.float32)
        a2 = 2.0 * lora_A[oh * O_SH:(oh + 1) * O_SH].T
        for base in (32, 64, 96):
            laug[base:base + R] = a2
        laug[0] = b[oh * O_SH:(oh + 1) * O_SH]
        laugs.append(laug.astype(bf16))

    in_maps = []
    for c in range(N_CORES):
        tg, oh = divmod(c, OH)
        in_maps.append({
            "xt": xts[tg],
            "wt": wts[oh],
            "bt": bt,
            "laug": laugs[oh],
        })
    return in_maps


def kernel(x, W, b, lora_A, lora_B):
    from concourse.bass_utils import run_bass_kernel_spmd

    nc = _get_nc()
    in_maps = _make_in_maps(dict(x=x, W=W, b=b, lora_A=lora_A, lora_B=lora_B))
    res = run_bass_kernel_spmd(nc, in_maps, core_ids=list(range(N_CORES)))
    out = np.empty((T, D_OUT), dtype=np.float32)
    for c in range(N_CORES):
        tg, oh = divmod(c, OH)
        out[tg * T_SH:(tg + 1) * T_SH, oh * O_SH:(oh + 1) * O_SH] = (
            res.results[c]["out"].astype(np.float32)
        )
    return out.reshape(B_DIM, S_DIM, D_OUT)


# revision 24
# speedup vs baseline: 1.9220x; 1.0003x over previous
"""LoRA Linear kernel for Trainium2, 8-core hybrid-parallel (4 token groups
x 2 out-feature halves).

out = x @ W^T + b + 2.0 * ((x @ lora_B^T) @ lora_A^T)

Per-core strategy (core = token-group tg x out-half oh):
  - Host marshals x^T and W^T shards pre-tiled in bf16 so every DMA is
    128 partitions x 8KB-contiguous and the kernel needs ZERO on-chip
    transposes. All matmuls bf16 (fp32 PSUM accumulate).
  - LoRA: xr^T = lora_B @ x^T computed once per t-strip (32 K=128 MMs)
    during the first o-strip pass; each output psum group then gets one
    extra K=17 matmul [xr^T; ones] @ [2*A^T; b] that adds BOTH the
    rank-16 update and the bias. No DVE work on any matmul's critical
    path.
  - Sharding 4 token-groups x 2 out-halves minimizes host->device
    traffic (~270MB vs 1.2GB for pure tensor-parallel).

Main loop: 4 o-strips (512) x 4 t-strips (512) x 4 t-tiles (128) x 32 k.
Output is written bf16 and upcast to fp32 on the host.
"""

import numpy as np

N_CORES = 8
B_DIM, S_DIM, D_IN, D_OUT = 4, 2048, 4096, 4096
T = B_DIM * S_DIM            # 8192 tokens
TG = 4                       # token groups
OH = 2                       # out-feature halves
T_SH = T // TG               # 2048 tokens per core
O_SH = D_OUT // OH           # 2048 out features per core
R = 16
P = 128
KB = D_IN // P               # 32 k-blocks
NOS = O_SH // 512            # 4 o-strips
NTS = T_SH // 512            # 4 t-strips
NSUB = 4                     # sub-DMAs per strip (8 k-blocks each)
KSUB = KB // NSUB

_CACHE = {}


def _build_nc():
    import concourse.bacc as bacc
    import concourse.mybir as mybir
    import concourse.tile as tile

    F32 = mybir.dt.float32
    BF16 = mybir.dt.bfloat16

    nc = bacc.Bacc(target_bir_lowering=False)
    # host-tiled layouts (see _make_in_maps):
    #   xt[ts*128+p, kb*512+u] = x_sh[ts*512+u, kb*128+p]   (= x^T tiled)
    #   wt[os*128+p, kb*512+u] = W_sh[os*512+u, kb*128+p]   (= W^T tiled)
    #   bt[p, kb*16+r]         = lora_B[r, kb*128+p]        (= B^T tiled)
    #   laug = [2*A_sh^T ; b_sh]  [17, O_SH]
    xt_d = nc.dram_tensor("xt", [NTS * P, KB * 512], BF16, kind="ExternalInput")
    wt_d = nc.dram_tensor("wt", [NOS * P, KB * 512], BF16, kind="ExternalInput")
    # laug is zero-padded to a full 128-partition operand: rows 32-47 /
    # 64-79 / 96-111 hold copies of 2*A^T (one per xr partial group), row 0
    # holds b. The lora matmul contracts all 128 rows, summing the three
    # xr partials and the bias in one shot.
    bt_d = nc.dram_tensor("bt", [P, KB * R], BF16, kind="ExternalInput")
    laug_d = nc.dram_tensor("laug", [P, O_SH], BF16, kind="ExternalInput")
    out_d = nc.dram_tensor("out", [T_SH, O_SH], BF16, kind="ExternalOutput")

    out_t = out_d[:].rearrange("(tt p) o -> p tt o", p=P)  # [128, 16, 2048]

    with tile.TileContext(nc) as tc:
        with (
            tc.tile_pool(name="const", bufs=1) as const,
            tc.tile_pool(name="xin", bufs=3) as xin,
            tc.tile_pool(name="win", bufs=2) as win,
            tc.tile_pool(name="osb", bufs=3) as osb_pool,
            tc.tile_pool(name="ps_o", bufs=4, space="PSUM") as ps_o,
            tc.tile_pool(name="ps_r", bufs=2, space="PSUM") as ps_r,
        ):
            btT = const.tile([P, KB, R], BF16)   # B^T tiled [128, 32, 16]
            laug = const.tile([P, O_SH], BF16)   # 2*A^T at rows 32/64/96+, b at 0
            xrT = const.tile([P, T_SH], BF16)    # xr partials at 32/64/96+, ones at 0

            # rows 32-47 / 64-79 / 96-111 get the three packed-xr partial
            # evictions; row 0 is the bias-ones row; the rest stay 0.
            nc.any.memset(xrT, 0.0)
            nc.any.memset(xrT[0:1, :], 1.0)
            nc.sync.dma_start(btT, bt_d[:].rearrange("p (kb r) -> p kb r", kb=KB))

            def x_sub(xsb, ts, s):
                nc.sync.dma_start(
                    xsb[:, s * KSUB:(s + 1) * KSUB, :],
                    xt_d[ts * P:(ts + 1) * P,
                         s * KSUB * 512:(s + 1) * KSUB * 512].rearrange(
                        "p (kb u) -> p kb u", kb=KSUB
                    ),
                )

            def w_sub(wsb, osi, s):
                nc.sync.dma_start(
                    wsb[:, s * KSUB:(s + 1) * KSUB, :],
                    wt_d[osi * P:(osi + 1) * P,
                         s * KSUB * 512:(s + 1) * KSUB * 512].rearrange(
                        "p (kb u) -> p kb u", kb=KSUB
                    ),
                )

            # startup: interleave the first x strip and first W strip so
            # the xr prologue (needs x+btT) and the first main groups
            # (need x+W) both start as soon as their sub-strips land.
            xsb0 = xin.tile([P, KB, 512], BF16, tag="x")
            wsb0 = win.tile([P, KB, 512], BF16, tag="w")
            for s in range(NSUB):
                x_sub(xsb0, 0, s)
                w_sub(wsb0, 0, s)
            nc.sync.dma_start(laug, laug_d[:])

            for osi in range(NOS):
                if osi == 0:
                    wsb = wsb0
                else:
                    wsb = win.tile([P, KB, 512], BF16, tag="w")
                    for s in range(NSUB):
                        w_sub(wsb, osi, s)
                for ts in range(NTS):
                    if osi == 0 and ts == 0:
                        xsb = xsb0
                    else:
                        xsb = xin.tile([P, KB, 512], BF16, tag="x")
                        for s in range(NSUB):
                            x_sub(xsb, ts, s)
                    if osi == 0:
                        # xr^T = B @ x^T, col-tiled 3x concurrent: partial
                        # sums over kb-thirds land at psum partition groups
                        # 32/64/96; the lora matmul's replicated 2*A^T rows
                        # absorb the cross-group reduction for free.
                        # group j takes kb = j, j+3, j+6, ... so step q only
                        # needs kbs 3q..3q+2 (consecutive -> sub-DMA local)
                        psr = ps_r.tile([P, 512], F32, tag="psr")
                        splits = [(j, 32 + 32 * j, list(range(j, KB, 3)))
                                  for j in range(3)]
                        for q in range(11):
                            for j, base, kbs in splits:
                                if q >= len(kbs):
                                    continue
                                kb = kbs[q]
                                nc.tensor.matmul(
                                    psr[base:base + R, :],
                                    btT[:, kb, :],
                                    xsb[:, kb, :],
                                    start=(q == 0),
                                    stop=(q == len(kbs) - 1),
                                    tile_position=(0, base),
                                )
                        for _, base, _ in splits:
                            nc.vector.tensor_copy(
                                out=xrT[base:base + R, ts * 512:(ts + 1) * 512],
                                in_=psr[base:base + R, :],
                            )
                    for tt in range(4):
                        pso = ps_o.tile([P, 512], F32, tag="pso")
                        for kb in range(KB):
                            nc.tensor.matmul(
                                pso,
                                xsb[:, kb, tt * P:(tt + 1) * P],
                                wsb[:, kb, :],
                                start=(kb == 0),
                                stop=False,
                            )
                        # rank-16 lora + bias in one full-array matmul
                        # (zero-padded K: rows 0-15 xr, row 32 ones/bias)
                        nc.tensor.matmul(
                            pso,
                            xrT[:, ts * 512 + tt * P:ts * 512 + (tt + 1) * P],
                            laug[:, osi * 512:(osi + 1) * 512],
                            start=False,
                            stop=True,
                        )
                        osb = osb_pool.tile([P, 512], BF16, tag="osb")
                        nc.vector.tensor_copy(out=osb, in_=pso)
                        nc.scalar.dma_start(
                            out_t[:, ts * 4 + tt, osi * 512:(osi + 1) * 512], osb
                        )

    nc.compile()
    return nc


def _get_nc():
    if "nc" not in _CACHE:
        _CACHE["nc"] = _build_nc()
    return _CACHE["nc"]


def _make_in_maps(inputs):
    import ml_dtypes

    bf16 = ml_dtypes.bfloat16
    x, W, b, lora_A, lora_B = (
        inputs["x"], inputs["W"], inputs["b"], inputs["lora_A"], inputs["lora_B"]
    )
    x_flat = np.asarray(x, dtype=np.float32).reshape(T, D_IN)
    W = np.asarray(W, dtype=np.float32)
    b = np.asarray(b, dtype=np.float32)
    lora_A = np.asarray(lora_A, dtype=np.float32)
    lora_B = np.asarray(lora_B, dtype=np.float32)

    # B^T tiled: bt[p, kb*16+r] = B[r, kb*128+p]
    bt = np.ascontiguousarray(
        lora_B.T.reshape(KB, P, R).transpose(1, 0, 2).reshape(P, KB * R)
    ).astype(bf16)

    xts = []
    for tg in range(TG):
        xs = x_flat[tg * T_SH:(tg + 1) * T_SH]           # [2048, 4096]
        h = xs.reshape(NTS, 512, KB, P).transpose(0, 3, 2, 1).astype(bf16)
        xts.append(np.ascontiguousarray(h.reshape(NTS * P, KB * 512)))
    wts, laugs = [], []
    for oh in range(OH):
        ws = W[oh * O_SH:(oh + 1) * O_SH]                # [2048, 4096]
        h = ws.reshape(NOS, 512, KB, P).transpose(0, 3, 2, 1).astype(bf16)
        wts.append(np.ascontiguousarray(h.reshape(NOS * P, KB * 512)))
        laug = np.zeros((P, O_SH), dtype=np.float32)
        a2 = 2.0 * lora_A[oh * O_SH:(oh + 1) * O_SH].T
        for base in (32, 64, 96):
            laug[base:base + R] = a2
        laug[0] = b[oh * O_SH:(oh + 1) * O_SH]
        laugs.append(laug.astype(bf16))

    in_maps = []
    for c in range(N_CORES):
        tg, oh = divmod(c, OH)
        in_maps.append({
            "xt": xts[tg],
            "wt": wts[oh],
            "bt": bt,
            "laug": laugs[oh],
        })
    return in_maps


def kernel(x, W, b, lora_A, lora_B):
    from concourse.bass_utils import run_bass_kernel_spmd

    nc = _get_nc()
    in_maps = _make_in_maps(dict(x=x, W=W, b=b, lora_A=lora_A, lora_B=lora_B))
    res = run_bass_kernel_spmd(nc, in_maps, core_ids=list(range(N_CORES)))
    out = np.empty((T, D_OUT), dtype=np.float32)
    for c in range(N_CORES):
        tg, oh = divmod(c, OH)
        out[tg * T_SH:(tg + 1) * T_SH, oh * O_SH:(oh + 1) * O_SH] = (
            res.results[c]["out"].astype(np.float32)
        )
    return out.reshape(B_DIM, S_DIM, D_OUT)


# revision 25
# speedup vs baseline: 1.9228x; 1.0004x over previous
"""LoRA Linear kernel for Trainium2, 8-core hybrid-parallel (4 token groups
x 2 out-feature halves).

out = x @ W^T + b + 2.0 * ((x @ lora_B^T) @ lora_A^T)

Per-core strategy (core = token-group tg x out-half oh):
  - Host marshals x^T and W^T shards pre-tiled in bf16 so every DMA is
    128 partitions x 8KB-contiguous and the kernel needs ZERO on-chip
    transposes. All matmuls bf16 (fp32 PSUM accumulate).
  - LoRA: xr^T = lora_B @ x^T computed once per t-strip (32 K=128 MMs)
    during the first o-strip pass; each output psum group then gets one
    extra K=17 matmul [xr^T; ones] @ [2*A^T; b] that adds BOTH the
    rank-16 update and the bias. No DVE work on any matmul's critical
    path.
  - Sharding 4 token-groups x 2 out-halves minimizes host->device
    traffic (~270MB vs 1.2GB for pure tensor-parallel).

Main loop: 4 o-strips (512) x 4 t-strips (512) x 4 t-tiles (128) x 32 k.
Output is written bf16 and upcast to fp32 on the host.
"""

import numpy as np

N_CORES = 8
B_DIM, S_DIM, D_IN, D_OUT = 4, 2048, 4096, 4096
T = B_DIM * S_DIM            # 8192 tokens
TG = 4                       # token groups
OH = 2                       # out-feature halves
T_SH = T // TG               # 2048 tokens per core
O_SH = D_OUT // OH           # 2048 out features per core
R = 16
P = 128
KB = D_IN // P               # 32 k-blocks
NOS = O_SH // 512            # 4 o-strips
NTS = T_SH // 512            # 4 t-strips
NSUB = 4                     # sub-DMAs per strip (8 k-blocks each)
KSUB = KB // NSUB

_CACHE = {}


def _build_nc():
    import concourse.bacc as bacc
    import concourse.mybir as mybir
    import concourse.tile as tile

    F32 = mybir.dt.float32
    BF16 = mybir.dt.bfloat16

    nc = bacc.Bacc(target_bir_lowering=False)
    # host-tiled layouts (see _make_in_maps):
    #   xt[ts*128+p, kb*512+u] = x_sh[ts*512+u, kb*128+p]   (= x^T tiled)
    #   wt[os*128+p, kb*512+u] = W_sh[os*512+u, kb*128+p]   (= W^T tiled)
    #   bt[p, kb*16+r]         = lora_B[r, kb*128+p]        (= B^T tiled)
    #   laug = [2*A_sh^T ; b_sh]  [17, O_SH]
    xt_d = nc.dram_tensor("xt", [NTS * P, KB * 512], BF16, kind="ExternalInput")
    wt_d = nc.dram_tensor("wt", [NOS * P, KB * 512], BF16, kind="ExternalInput")
    # laug is zero-padded to a full 128-partition operand: rows 32-47 /
    # 64-79 / 96-111 hold copies of 2*A^T (one per xr partial group), row 0
    # holds b. The lora matmul contracts all 128 rows, summing the three
    # xr partials and the bias in one shot.
    bt_d = nc.dram_tensor("bt", [P, KB * R], BF16, kind="ExternalInput")
    laug_d = nc.dram_tensor("laug", [P, O_SH], BF16, kind="ExternalInput")
    out_d = nc.dram_tensor("out", [T_SH, O_SH], BF16, kind="ExternalOutput")

    out_t = out_d[:].rearrange("(tt p) o -> p tt o", p=P)  # [128, 16, 2048]

    with tile.TileContext(nc) as tc:
        with (
            tc.tile_pool(name="const", bufs=1) as const,
            tc.tile_pool(name="xin", bufs=4) as xin,
            tc.tile_pool(name="win", bufs=2) as win,
            tc.tile_pool(name="osb", bufs=3) as osb_pool,
            tc.tile_pool(name="ps_o", bufs=5, space="PSUM") as ps_o,
            tc.tile_pool(name="ps_r", bufs=1, space="PSUM") as ps_r,
        ):
            btT = const.tile([P, KB, R], BF16)   # B^T tiled [128, 32, 16]
            laug = const.tile([P, O_SH], BF16)   # 2*A^T at rows 32/64/96+, b at 0
            xrT = const.tile([P, T_SH], BF16)    # xr partials at 32/64/96+, ones at 0

            # rows 32-47 / 64-79 / 96-111 get the three packed-xr partial
            # evictions; row 0 is the bias-ones row; the rest stay 0.
            nc.any.memset(xrT, 0.0)
            nc.any.memset(xrT[0:1, :], 1.0)
            nc.sync.dma_start(btT, bt_d[:].rearrange("p (kb r) -> p kb r", kb=KB))

            def x_sub(xsb, ts, s):
                nc.sync.dma_start(
                    xsb[:, s * KSUB:(s + 1) * KSUB, :],
                    xt_d[ts * P:(ts + 1) * P,
                         s * KSUB * 512:(s + 1) * KSUB * 512].rearrange(
                        "p (kb u) -> p kb u", kb=KSUB
                    ),
                )

            def w_sub(wsb, osi, s):
                nc.sync.dma_start(
                    wsb[:, s * KSUB:(s + 1) * KSUB, :],
                    wt_d[osi * P:(osi + 1) * P,
                         s * KSUB * 512:(s + 1) * KSUB * 512].rearrange(
                        "p (kb u) -> p kb u", kb=KSUB
                    ),
                )

            # startup: interleave the first x strip and first W strip so
            # the xr prologue (needs x+btT) and the first main groups
            # (need x+W) both start as soon as their sub-strips land.
            xsb0 = xin.tile([P, KB, 512], BF16, tag="x")
            wsb0 = win.tile([P, KB, 512], BF16, tag="w")
            for s in range(NSUB):
                x_sub(xsb0, 0, s)
                w_sub(wsb0, 0, s)
            nc.sync.dma_start(laug, laug_d[:])

            for osi in range(NOS):
                if osi == 0:
                    wsb = wsb0
                else:
                    wsb = win.tile([P, KB, 512], BF16, tag="w")
                    for s in range(NSUB):
                        w_sub(wsb, osi, s)
                for ts in range(NTS):
                    if osi == 0 and ts == 0:
                        xsb = xsb0
                    else:
                        xsb = xin.tile([P, KB, 512], BF16, tag="x")
                        for s in range(NSUB):
                            x_sub(xsb, ts, s)
                    if osi == 0:
                        # xr^T = B @ x^T, col-tiled 3x concurrent: partial
                        # sums over kb-thirds land at psum partition groups
                        # 32/64/96; the lora matmul's replicated 2*A^T rows
                        # absorb the cross-group reduction for free.
                        # group j takes kb = j, j+3, j+6, ... so step q only
                        # needs kbs 3q..3q+2 (consecutive -> sub-DMA local)
                        psr = ps_r.tile([P, 512], F32, tag="psr")
                        splits = [(j, 32 + 32 * j, list(range(j, KB, 3)))
                                  for j in range(3)]
                        for q in range(11):
                            for j, base, kbs in splits:
                                if q >= len(kbs):
                                    continue
                                kb = kbs[q]
                                nc.tensor.matmul(
                                    psr[base:base + R, :],
                                    btT[:, kb, :],
                                    xsb[:, kb, :],
                                    start=(q == 0),
                                    stop=(q == len(kbs) - 1),
                                    tile_position=(0, base),
                                )
                        for _, base, _ in splits:
                            nc.vector.tensor_copy(
                                out=xrT[base:base + R, ts * 512:(ts + 1) * 512],
                                in_=psr[base:base + R, :],
                            )
                    for tt in range(4):
                        pso = ps_o.tile([P, 512], F32, tag="pso")
                        for kb in range(KB):
                            nc.tensor.matmul(
                                pso,
                                xsb[:, kb, tt * P:(tt + 1) * P],
                                wsb[:, kb, :],
                                start=(kb == 0),
                                stop=False,
                            )
                        # rank-16 lora + bias in one full-array matmul
                        # (zero-padded K: rows 0-15 xr, row 32 ones/bias)
                        nc.tensor.matmul(
                            pso,
                            xrT[:, ts * 512 + tt * P:ts * 512 + (tt + 1) * P],
                            laug[:, osi * 512:(osi + 1) * 512],
                            start=False,
                            stop=True,
                        )
                        osb = osb_pool.tile([P, 512], BF16, tag="osb")
                        nc.vector.tensor_copy(out=osb, in_=pso)
                        nc.scalar.dma_start(
                            out_t[:, ts * 4 + tt, osi * 512:(osi + 1) * 512], osb
                        )

    nc.compile()
    return nc


def _get_nc():
    if "nc" not in _CACHE:
        _CACHE["nc"] = _build_nc()
    return _CACHE["nc"]


def _make_in_maps(inputs):
    import ml_dtypes

    bf16 = ml_dtypes.bfloat16
    x, W, b, lora_A, lora_B = (
        inputs["x"], inputs["W"], inputs["b"], inputs["lora_A"], inputs["lora_B"]
    )
    x_flat = np.asarray(x, dtype=np.float32).reshape(T, D_IN)
    W = np.asarray(W, dtype=np.float32)
    b = np.asarray(b, dtype=np.float32)
    lora_A = np.asarray(lora_A, dtype=np.float32)
    lora_B = np.asarray(lora_B, dtype=np.float32)

    # B^T tiled: bt[p, kb*16+r] = B[r, kb*128+p]
    bt = np.ascontiguousarray(
        lora_B.T.reshape(KB, P, R).transpose(1, 0, 2).reshape(P, KB * R)
    ).astype(bf16)

    xts = []
    for tg in range(TG):
        xs = x_flat[tg * T_SH:(tg + 1) * T_SH]           # [2048, 4096]
        h = xs.reshape(NTS, 512, KB, P).transpose(0, 3, 2, 1).astype(bf16)
        xts.append(np.ascontiguousarray(h.reshape(NTS * P, KB * 512)))
    wts, laugs = [], []
    for oh in range(OH):
        ws = W[oh * O_SH:(oh + 1) * O_SH]                # [2048, 4096]
        h = ws.reshape(NOS, 512, KB, P).transpose(0, 3, 2, 1).astype(bf16)
        wts.append(np.ascontiguousarray(h.reshape(NOS * P, KB * 512)))
        laug = np.zeros((P, O_SH), dtype=np.float32)
        a2 = 2.0 * lora_A[oh * O_SH:(oh + 1) * O_SH].T
        for base in (32, 64, 96):
            laug[base:base + R] = a2
        laug[0] = b[oh * O_SH:(oh + 1) * O_SH]
        laugs.append(laug.astype(bf16))

    in_maps = []
    for c in range(N_CORES):
        tg, oh = divmod(c, OH)
        in_maps.append({
            "xt": xts[tg],
            "wt": wts[oh],
            "bt": bt,
            "laug": laugs[oh],
        })
    return in_maps


def kernel(x, W, b, lora_A, lora_B):
    from concourse.bass_utils import run_bass_kernel_spmd

    nc = _get_nc()
    in_maps = _make_in_maps(dict(x=x, W=W, b=b, lora_A=lora_A, lora_B=lora_B))
    res = run_bass_kernel_spmd(nc, in_maps, core_ids=list(range(N_CORES)))
    out = np.empty((T, D_OUT), dtype=np.float32)
    for c in range(N_CORES):
        tg, oh = divmod(c, OH)
        out[tg * T_SH:(tg + 1) * T_SH, oh * O_SH:(oh + 1) * O_SH] = (
            res.results[c]["out"].astype(np.float32)
        )
    return out.reshape(B_DIM, S_DIM, D_OUT)
